# revision 8
# baseline (speedup 1.0000x reference)
"""Trainium2 Bass kernel for nn_ObservationEmbeddingV2 (grouped SwiGLU -> RMSNorm -> +term_embed).

Contract: kernel(**inputs) takes FULL unsharded inputs (numpy), returns FULL output.
Sharding: data-parallel over batch across 8 NeuronCores (2048 rows each); the small
per-group weights, norm params and term embedding are replicated.

Device dataflow (per core, feature-major activations):
  xT [2048 feat, 2048 batch] (host-pre-transposed shard of obs)
  for g in 7 groups:                       # g0: K_in=512, rest: K_in=256
    w1T/w3T [K_in, 2048], w2T [2048, 1024] resident in SBUF (host-pre-transposed)
    for sblk in 4 x 512 batch cols:
      L1: gateT/valT [h_tile=128, 512] = accum_k MM(w1T/w3T slice, xT k-tile)
          silu(gate) in-place in PSUM (ACT), act = gate*val -> SBUF (DVE)
      L2: tok [b_tile=128, d_half=512] = accum_ht MM(actT slice, w2T slice)
      RMSNorm over d (sumsq via DVE reduce from PSUM, sqrt/recip, scale),
      * norm_w + (norm_b + te[g]), DMA out rows -> out[b, g, :]
All matmuls run as float32r (fp32 storage, full-rate PE path).
"""

import sys

import numpy as np

sys.path.insert(0, "/opt/trn_rl_repo")

import concourse.bass as bass  # noqa: E402
import concourse.bacc as bacc  # noqa: E402
import concourse.tile as tile  # noqa: E402
from concourse import mybir  # noqa: E402
from concourse import bass_utils  # noqa: E402
from contextlib import ExitStack  # noqa: E402

N_CORES = 8
B = 16384
B_LOCAL = B // N_CORES  # 2048
D = 1024
H = 2048
SBLK = 512              # batch columns per superblock
NSBLK = B_LOCAL // SBLK  # 4
NHT = H // 128           # 16
EPS = 1e-5
# (feature offset in obs, K_in) per group; g0 is the concat [t0,t1] group
GROUPS = [(0, 512)] + [(512 + i * 256, 256) for i in range(6)]

F32 = mybir.dt.float32
F32R = mybir.dt.float32r
MM_DT = F32R  # full-rate fp32 PE path
AF = mybir.ActivationFunctionType
ALU = mybir.AluOpType

_nc_cache = {}


def _build(normw_is_one=True, normb_is_zero=True):
    nc = bacc.Bacc("TRN2", target_bir_lowering=False, debug=False, enable_asserts=False)

    xt_d = nc.dram_tensor("xt", [2048, B_LOCAL], F32R, kind="ExternalInput").ap()
    w13_d = nc.dram_tensor("w13t", [4096, H], F32R, kind="ExternalInput").ap()
    w2_d = nc.dram_tensor("w2t", [7 * H, D], F32R, kind="ExternalInput").ap()
    nw_d = nc.dram_tensor("normw", [D], F32, kind="ExternalInput").ap()
    nb_d = nc.dram_tensor("normb", [D], F32, kind="ExternalInput").ap()
    te_d = nc.dram_tensor("te", [7, D], F32, kind="ExternalInput").ap()
    out_d = nc.dram_tensor("out", [B_LOCAL, 7, D], F32, kind="ExternalOutput").ap()

    with tile.TileContext(nc) as tc, ExitStack() as ctx:
        w13_pool = ctx.enter_context(tc.tile_pool(name="w13", bufs=8))
        w2_pool = ctx.enter_context(tc.tile_pool(name="w2", bufs=16))
        xt_pool = ctx.enter_context(tc.tile_pool(name="xt", bufs=4))
        act_pool = ctx.enter_context(tc.tile_pool(name="act", bufs=16))
        tok_pool = ctx.enter_context(tc.tile_pool(name="tok", bufs=2))
        sq_pool = ctx.enter_context(tc.tile_pool(name="sq", bufs=1))
        sg_pool = ctx.enter_context(tc.tile_pool(name="sg", bufs=2))
        ot_pool = ctx.enter_context(tc.tile_pool(name="ot", bufs=2))
        te_pool = ctx.enter_context(tc.tile_pool(name="teb", bufs=1))
        tetmp_pool = ctx.enter_context(tc.tile_pool(name="tetmp", bufs=1))
        small_pool = ctx.enter_context(tc.tile_pool(name="small", bufs=4))
        const_pool = ctx.enter_context(tc.tile_pool(name="consts", bufs=1))
        psA = ctx.enter_context(tc.tile_pool(name="psA", bufs=4, space="PSUM"))
        psB = ctx.enter_context(tc.tile_pool(name="psB", bufs=4, space="PSUM"))

        normw_b = None
        if not normw_is_one:
            normw_b = const_pool.tile([128, D], F32, tag="normw")
            nc.sync.dma_start(out=normw_b, in_=nw_d.partition_broadcast(128))
        normb_b = None
        if not normb_is_zero:
            normb_b = const_pool.tile([128, D], F32, tag="normb")
            nc.sync.dma_start(out=normb_b, in_=nb_d.partition_broadcast(128))
        eps_t = const_pool.tile([128, 1], F32, tag="eps")
        nc.vector.memset(eps_t, EPS)

        w13_row = 0
        for g, (foff, kin) in enumerate(GROUPS):
            nk = kin // 128

            w1T = []
            w3T = []
            for k in range(nk):
                t = w13_pool.tile([128, H], MM_DT, tag="w13")
                nc.sync.dma_start(
                    out=t, in_=w13_d[w13_row + k * 128 : w13_row + (k + 1) * 128, :]
                )
                w1T.append(t)
            for k in range(nk):
                t = w13_pool.tile([128, H], MM_DT, tag="w13")
                nc.sync.dma_start(
                    out=t,
                    in_=w13_d[w13_row + kin + k * 128 : w13_row + kin + (k + 1) * 128, :],
                )
                w3T.append(t)
            w13_row += 2 * kin

            w2T = []
            for ht in range(NHT):
                t = w2_pool.tile([128, D], MM_DT, tag="w2")
                nc.sync.dma_start(
                    out=t, in_=w2_d[g * H + ht * 128 : g * H + (ht + 1) * 128, :]
                )
                w2T.append(t)

            bias_te = te_pool.tile([128, D], F32, tag="biaste")
            if normb_is_zero:
                nc.sync.dma_start(out=bias_te, in_=te_d[g].partition_broadcast(128))
            else:
                te_tmp = tetmp_pool.tile([128, D], F32, tag="tetmp")
                nc.sync.dma_start(out=te_tmp, in_=te_d[g].partition_broadcast(128))
                nc.vector.tensor_add(bias_te, te_tmp, normb_b)

            for sblk in range(NSBLK):
                b0 = sblk * SBLK
                xts = []
                for k in range(nk):
                    t = xt_pool.tile([128, SBLK], MM_DT, tag="xt")
                    nc.sync.dma_start(
                        out=t,
                        in_=xt_d[foff + k * 128 : foff + (k + 1) * 128, b0 : b0 + SBLK],
                    )
                    xts.append(t)

                # L1: SwiGLU up-projections, feature-major output
                actT = []
                for ht in range(NHT):
                    hs = slice(ht * 128, (ht + 1) * 128)
                    pg = psA.tile([128, SBLK], F32, tag="pg")
                    pv = psA.tile([128, SBLK], F32, tag="pg")
                    for k in range(nk):
                        nc.tensor.matmul(
                            pg,
                            w1T[k][:, hs],
                            xts[k],
                            start=(k == 0),
                            stop=(k == nk - 1),
                        )
                    for k in range(nk):
                        nc.tensor.matmul(
                            pv,
                            w3T[k][:, hs],
                            xts[k],
                            start=(k == 0),
                            stop=(k == nk - 1),
                        )
                    sg = sg_pool.tile([128, SBLK], F32, tag="sg")
                    nc.scalar.activation(sg, pg, AF.Silu)
                    a = act_pool.tile([128, SBLK], MM_DT, tag="act")
                    nc.vector.tensor_mul(a, sg, pv)
                    actT.append(a)

                # L2: down-projection to batch-major tok, fused RMSNorm + embed
                for bt in range(SBLK // 128):
                    bs = slice(bt * 128, (bt + 1) * 128)
                    parts = small_pool.tile([128, 2], F32, tag="parts")
                    tok = tok_pool.tile([128, D], F32, tag="tok")
                    for dh in range(2):
                        ds = slice(dh * 512, (dh + 1) * 512)
                        pt = psB.tile([128, 512], F32, tag="pt")
                        for ht in range(NHT):
                            nc.tensor.matmul(
                                pt,
                                actT[ht][:, bs],
                                w2T[ht][:, ds],
                                start=(ht == 0),
                                stop=(ht == NHT - 1),
                            )
                        sq = sq_pool.tile([128, 512], F32, tag="sq")
                        nc.scalar.activation(
                            sq, pt, AF.Square, accum_out=parts[:, dh : dh + 1]
                        )
                        nc.vector.tensor_copy(out=tok[:, ds], in_=pt)
                    ssum = small_pool.tile([128, 1], F32, tag="ssum")
                    nc.vector.tensor_add(ssum, parts[:, 0:1], parts[:, 1:2])
                    rms = small_pool.tile([128, 1], F32, tag="rms")
                    nc.scalar.activation(rms, ssum, AF.Sqrt, bias=eps_t[:, 0:1], scale=1.0 / D)
                    rstd = small_pool.tile([128, 1], F32, tag="rstd")
                    nc.vector.reciprocal(rstd, rms)
                    ot = ot_pool.tile([128, D], F32, tag="ot")
                    nc.scalar.activation(ot, tok, AF.Copy, scale=rstd[:, 0:1])
                    if normw_b is not None:
                        nc.vector.tensor_mul(ot, ot, normw_b)
                    nc.vector.tensor_add(ot, ot, bias_te)
                    nc.sync.dma_start(out=out_d[b0 + bt * 128 : b0 + (bt + 1) * 128, g, :], in_=ot)

    nc.compile()
    return nc


def _get_nc(normw_is_one, normb_is_zero):
    key = (normw_is_one, normb_is_zero)
    if key not in _nc_cache:
        _nc_cache[key] = _build(*key)
    return _nc_cache[key]


def _host_pack(inputs):
    f = np.float32
    obs = np.asarray(inputs["obs"], f)
    w1_g0 = np.asarray(inputs["w1_g0"], f)
    w3_g0 = np.asarray(inputs["w3_g0"], f)
    w2_g0 = np.asarray(inputs["w2_g0"], f)
    w1_r = np.asarray(inputs["w1_r"], f)
    w3_r = np.asarray(inputs["w3_r"], f)
    w2_r = np.asarray(inputs["w2_r"], f)

    w13_parts = [w1_g0.T, w3_g0.T]
    for i in range(6):
        w13_parts.append(w1_r[i].T)
        w13_parts.append(w3_r[i].T)
    w13t = np.ascontiguousarray(np.concatenate(w13_parts, axis=0))  # [4096, 2048]

    w2_parts = [w2_g0.T] + [w2_r[i].T for i in range(6)]
    w2t = np.ascontiguousarray(np.concatenate(w2_parts, axis=0))  # [14336, 1024]

    common = {
        "w13t": w13t,
        "w2t": w2t,
        "normw": np.ascontiguousarray(np.asarray(inputs["norm_w"], f)),
        "normb": np.ascontiguousarray(np.asarray(inputs["norm_b"], f)),
        "te": np.ascontiguousarray(np.asarray(inputs["term_embed"], f)),
    }
    in_maps = []
    for c in range(N_CORES):
        m = dict(common)
        m["xt"] = np.ascontiguousarray(obs[c * B_LOCAL : (c + 1) * B_LOCAL].T)
        in_maps.append(m)
    return in_maps


def run(inputs, trace=False, **kw):
    normw_is_one = bool(np.all(np.asarray(inputs["norm_w"]) == 1.0))
    normb_is_zero = bool(np.all(np.asarray(inputs["norm_b"]) == 0.0))
    nc = _get_nc(normw_is_one, normb_is_zero)
    in_maps = _host_pack(inputs)
    res = bass_utils.run_bass_kernel_spmd(
        nc, in_maps, core_ids=list(range(N_CORES)), trace=trace, **kw
    )
    out = np.concatenate([r["out"] for r in res.results], axis=0)
    return out, res


def kernel(**inputs):
    out, _ = run(inputs, trace=False)
    return out


# revision 9
# speedup vs baseline: 1.0061x; 1.0061x over previous
"""Trainium2 Bass kernel for nn_ObservationEmbeddingV2 (grouped SwiGLU -> RMSNorm -> +term_embed).

Contract: kernel(**inputs) takes FULL unsharded inputs (numpy), returns FULL output.
Sharding: data-parallel over batch across 8 NeuronCores (2048 rows each); the small
per-group weights, norm params and term embedding are replicated.

Device dataflow (per core, feature-major activations):
  xT [2048 feat, 2048 batch] (host-pre-transposed shard of obs)
  for g in 7 groups:                       # g0: K_in=512, rest: K_in=256
    w1T/w3T [K_in, 2048], w2T [2048, 1024] resident in SBUF (host-pre-transposed)
    for sblk in 4 x 512 batch cols:
      L1: gateT/valT [h_tile=128, 512] = accum_k MM(w1T/w3T slice, xT k-tile)
          silu(gate) in-place in PSUM (ACT), act = gate*val -> SBUF (DVE)
      L2: tok [b_tile=128, d_half=512] = accum_ht MM(actT slice, w2T slice)
      RMSNorm over d (sumsq via DVE reduce from PSUM, sqrt/recip, scale),
      * norm_w + (norm_b + te[g]), DMA out rows -> out[b, g, :]
All matmuls run as float32r (fp32 storage, full-rate PE path).
"""

import sys

import numpy as np

sys.path.insert(0, "/opt/trn_rl_repo")

import concourse.bass as bass  # noqa: E402
import concourse.bacc as bacc  # noqa: E402
import concourse.tile as tile  # noqa: E402
from concourse import mybir  # noqa: E402
from concourse import bass_utils  # noqa: E402
from contextlib import ExitStack  # noqa: E402

N_CORES = 8
B = 16384
B_LOCAL = B // N_CORES  # 2048
D = 1024
H = 2048
SBLK = 512              # batch columns per superblock
NSBLK = B_LOCAL // SBLK  # 4
NHT = H // 128           # 16
EPS = 1e-5
# (feature offset in obs, K_in) per group; g0 is the concat [t0,t1] group
GROUPS = [(0, 512)] + [(512 + i * 256, 256) for i in range(6)]

F32 = mybir.dt.float32
F32R = mybir.dt.float32r
MM_DT = F32R  # full-rate fp32 PE path
AF = mybir.ActivationFunctionType
ALU = mybir.AluOpType

_nc_cache = {}


def _build(normw_is_one=True, normb_is_zero=True):
    nc = bacc.Bacc("TRN2", target_bir_lowering=False, debug=False, enable_asserts=False)

    xt_d = nc.dram_tensor("xt", [2048, B_LOCAL], F32R, kind="ExternalInput").ap()
    w13_d = nc.dram_tensor("w13t", [4096, H], F32R, kind="ExternalInput").ap()
    w2_d = nc.dram_tensor("w2t", [7 * H, D], F32R, kind="ExternalInput").ap()
    nw_d = nc.dram_tensor("normw", [D], F32, kind="ExternalInput").ap()
    nb_d = nc.dram_tensor("normb", [D], F32, kind="ExternalInput").ap()
    te_d = nc.dram_tensor("te", [7, D], F32, kind="ExternalInput").ap()
    out_d = nc.dram_tensor("out", [B_LOCAL, 7, D], F32, kind="ExternalOutput").ap()

    with tile.TileContext(nc) as tc, ExitStack() as ctx:
        w13_pool = ctx.enter_context(tc.tile_pool(name="w13", bufs=8))
        w2_pool = ctx.enter_context(tc.tile_pool(name="w2", bufs=16))
        xt_pool = ctx.enter_context(tc.tile_pool(name="xt", bufs=2))
        act_pool = ctx.enter_context(tc.tile_pool(name="act", bufs=16))
        tok_pool = ctx.enter_context(tc.tile_pool(name="tok", bufs=2))
        sq_pool = ctx.enter_context(tc.tile_pool(name="sq", bufs=1))
        sg_pool = ctx.enter_context(tc.tile_pool(name="sg", bufs=2))
        ot_pool = ctx.enter_context(tc.tile_pool(name="ot", bufs=2))
        te_pool = ctx.enter_context(tc.tile_pool(name="teb", bufs=1))
        tetmp_pool = ctx.enter_context(tc.tile_pool(name="tetmp", bufs=1))
        small_pool = ctx.enter_context(tc.tile_pool(name="small", bufs=4))
        const_pool = ctx.enter_context(tc.tile_pool(name="consts", bufs=1))
        psA = ctx.enter_context(tc.tile_pool(name="psA", bufs=4, space="PSUM"))
        psB = ctx.enter_context(tc.tile_pool(name="psB", bufs=4, space="PSUM"))

        normw_b = None
        if not normw_is_one:
            normw_b = const_pool.tile([128, D], F32, tag="normw")
            nc.sync.dma_start(out=normw_b, in_=nw_d.partition_broadcast(128))
        normb_b = None
        if not normb_is_zero:
            normb_b = const_pool.tile([128, D], F32, tag="normb")
            nc.sync.dma_start(out=normb_b, in_=nb_d.partition_broadcast(128))
        eps_t = const_pool.tile([128, 1], F32, tag="eps")
        nc.vector.memset(eps_t, EPS)

        # first xt load issued ahead of all weight DMAs so PE can start early
        pre0 = xt_pool.tile([128, 4, SBLK], MM_DT, tag="xt")
        nc.sync.dma_start(
            out=pre0,
            in_=xt_d[0:512, 0:SBLK].rearrange("(t p) b -> p t b", p=128),
        )
        pre_xts0 = [pre0[:, k, :] for k in range(4)]

        w13_row = 0
        for g, (foff, kin) in enumerate(GROUPS):
            nk = kin // 128

            w1T = []
            w3T = []
            for k in range(nk):
                t = w13_pool.tile([128, H], MM_DT, tag="w13")
                nc.sync.dma_start(
                    out=t, in_=w13_d[w13_row + k * 128 : w13_row + (k + 1) * 128, :]
                )
                w1T.append(t)
            for k in range(nk):
                t = w13_pool.tile([128, H], MM_DT, tag="w13")
                nc.sync.dma_start(
                    out=t,
                    in_=w13_d[w13_row + kin + k * 128 : w13_row + kin + (k + 1) * 128, :],
                )
                w3T.append(t)
            w13_row += 2 * kin

            w2T = []
            for ht in range(NHT):
                t = w2_pool.tile([128, D], MM_DT, tag="w2")
                nc.gpsimd.dma_start(
                    out=t, in_=w2_d[g * H + ht * 128 : g * H + (ht + 1) * 128, :]
                )
                w2T.append(t)

            bias_te = te_pool.tile([128, D], F32, tag="biaste")
            if normb_is_zero:
                nc.gpsimd.dma_start(out=bias_te, in_=te_d[g].partition_broadcast(128))
            else:
                te_tmp = tetmp_pool.tile([128, D], F32, tag="tetmp")
                nc.gpsimd.dma_start(out=te_tmp, in_=te_d[g].partition_broadcast(128))
                nc.vector.tensor_add(bias_te, te_tmp, normb_b)

            def load_xts(sblk):
                b0 = sblk * SBLK
                t = xt_pool.tile([128, nk, SBLK], MM_DT, tag="xt")
                nc.sync.dma_start(
                    out=t,
                    in_=xt_d[foff : foff + kin, b0 : b0 + SBLK].rearrange(
                        "(t p) b -> p t b", p=128
                    ),
                )
                return [t[:, k, :] for k in range(nk)]

            if g == 0:
                xts_cur = pre_xts0
            else:
                xts_cur = load_xts(0)
            for sblk in range(NSBLK):
                b0 = sblk * SBLK
                xts = xts_cur
                xts_next = load_xts(sblk + 1) if sblk + 1 < NSBLK else None

                # L1: SwiGLU up-projections, feature-major output
                actT = []
                for ht in range(NHT):
                    hs = slice(ht * 128, (ht + 1) * 128)
                    pg = psA.tile([128, SBLK], F32, tag="pg")
                    pv = psA.tile([128, SBLK], F32, tag="pg")
                    for k in range(nk):
                        nc.tensor.matmul(
                            pg,
                            w1T[k][:, hs],
                            xts[k],
                            start=(k == 0),
                            stop=(k == nk - 1),
                        )
                    for k in range(nk):
                        nc.tensor.matmul(
                            pv,
                            w3T[k][:, hs],
                            xts[k],
                            start=(k == 0),
                            stop=(k == nk - 1),
                        )
                    sg = sg_pool.tile([128, SBLK], F32, tag="sg")
                    nc.scalar.activation(sg, pg, AF.Silu)
                    a = act_pool.tile([128, SBLK], MM_DT, tag="act")
                    nc.vector.tensor_mul(a, sg, pv)
                    actT.append(a)

                # L2: down-projection to batch-major tok, fused RMSNorm + embed
                for bt in range(SBLK // 128):
                    bs = slice(bt * 128, (bt + 1) * 128)
                    parts = small_pool.tile([128, 2], F32, tag="parts")
                    tok = tok_pool.tile([128, D], F32, tag="tok")
                    for dh in range(2):
                        ds = slice(dh * 512, (dh + 1) * 512)
                        pt = psB.tile([128, 512], F32, tag="pt")
                        for ht in range(NHT):
                            nc.tensor.matmul(
                                pt,
                                actT[ht][:, bs],
                                w2T[ht][:, ds],
                                start=(ht == 0),
                                stop=(ht == NHT - 1),
                            )
                        sq = sq_pool.tile([128, 512], F32, tag="sq")
                        nc.scalar.activation(
                            sq, pt, AF.Square, accum_out=parts[:, dh : dh + 1]
                        )
                        nc.vector.tensor_copy(out=tok[:, ds], in_=pt)
                    ssum = small_pool.tile([128, 1], F32, tag="ssum")
                    nc.vector.tensor_add(ssum, parts[:, 0:1], parts[:, 1:2])
                    rms = small_pool.tile([128, 1], F32, tag="rms")
                    nc.scalar.activation(rms, ssum, AF.Sqrt, bias=eps_t[:, 0:1], scale=1.0 / D)
                    rstd = small_pool.tile([128, 1], F32, tag="rstd")
                    nc.vector.reciprocal(rstd, rms)
                    ot = ot_pool.tile([128, D], F32, tag="ot")
                    nc.scalar.activation(ot, tok, AF.Copy, scale=rstd[:, 0:1])
                    if normw_b is not None:
                        nc.vector.tensor_mul(ot, ot, normw_b)
                    nc.vector.tensor_add(ot, ot, bias_te)
                    nc.gpsimd.dma_start(out=out_d[b0 + bt * 128 : b0 + (bt + 1) * 128, g, :], in_=ot)
                xts_cur = xts_next

    nc.compile()
    return nc


def _get_nc(normw_is_one, normb_is_zero):
    key = (normw_is_one, normb_is_zero)
    if key not in _nc_cache:
        _nc_cache[key] = _build(*key)
    return _nc_cache[key]


def _host_pack(inputs):
    f = np.float32
    obs = np.asarray(inputs["obs"], f)
    w1_g0 = np.asarray(inputs["w1_g0"], f)
    w3_g0 = np.asarray(inputs["w3_g0"], f)
    w2_g0 = np.asarray(inputs["w2_g0"], f)
    w1_r = np.asarray(inputs["w1_r"], f)
    w3_r = np.asarray(inputs["w3_r"], f)
    w2_r = np.asarray(inputs["w2_r"], f)

    w13_parts = [w1_g0.T, w3_g0.T]
    for i in range(6):
        w13_parts.append(w1_r[i].T)
        w13_parts.append(w3_r[i].T)
    w13t = np.ascontiguousarray(np.concatenate(w13_parts, axis=0))  # [4096, 2048]

    w2_parts = [w2_g0.T] + [w2_r[i].T for i in range(6)]
    w2t = np.ascontiguousarray(np.concatenate(w2_parts, axis=0))  # [14336, 1024]

    common = {
        "w13t": w13t,
        "w2t": w2t,
        "normw": np.ascontiguousarray(np.asarray(inputs["norm_w"], f)),
        "normb": np.ascontiguousarray(np.asarray(inputs["norm_b"], f)),
        "te": np.ascontiguousarray(np.asarray(inputs["term_embed"], f)),
    }
    in_maps = []
    for c in range(N_CORES):
        m = dict(common)
        m["xt"] = np.ascontiguousarray(obs[c * B_LOCAL : (c + 1) * B_LOCAL].T)
        in_maps.append(m)
    return in_maps


def run(inputs, trace=False, **kw):
    normw_is_one = bool(np.all(np.asarray(inputs["norm_w"]) == 1.0))
    normb_is_zero = bool(np.all(np.asarray(inputs["norm_b"]) == 0.0))
    nc = _get_nc(normw_is_one, normb_is_zero)
    in_maps = _host_pack(inputs)
    res = bass_utils.run_bass_kernel_spmd(
        nc, in_maps, core_ids=list(range(N_CORES)), trace=trace, **kw
    )
    out = np.concatenate([r["out"] for r in res.results], axis=0)
    return out, res


def kernel(**inputs):
    out, _ = run(inputs, trace=False)
    return out


# revision 10
# speedup vs baseline: 1.0286x; 1.0224x over previous
"""Trainium2 Bass kernel for nn_ObservationEmbeddingV2 (grouped SwiGLU -> RMSNorm -> +term_embed).

Contract: kernel(**inputs) takes FULL unsharded inputs (numpy), returns FULL output.
Sharding: data-parallel over batch across 8 NeuronCores (2048 rows each); the small
per-group weights, norm params and term embedding are replicated.

Device dataflow (per core, feature-major activations):
  xT [2048 feat, 2048 batch] (host-pre-transposed shard of obs)
  for g in 7 groups:                       # g0: K_in=512, rest: K_in=256
    w1T/w3T [K_in, 2048], w2T [2048, 1024] resident in SBUF (host-pre-transposed)
    for sblk in 4 x 512 batch cols:
      L1: gateT/valT [h_tile=128, 512] = accum_k MM(w1T/w3T slice, xT k-tile)
          silu(gate) in-place in PSUM (ACT), act = gate*val -> SBUF (DVE)
      L2: tok [b_tile=128, d_half=512] = accum_ht MM(actT slice, w2T slice)
      RMSNorm over d (sumsq via DVE reduce from PSUM, sqrt/recip, scale),
      * norm_w + (norm_b + te[g]), DMA out rows -> out[b, g, :]
All matmuls run as float32r (fp32 storage, full-rate PE path).
"""

import sys

import numpy as np

sys.path.insert(0, "/opt/trn_rl_repo")

import concourse.bass as bass  # noqa: E402
import concourse.bacc as bacc  # noqa: E402
import concourse.tile as tile  # noqa: E402
from concourse import mybir  # noqa: E402
from concourse import bass_utils  # noqa: E402
from contextlib import ExitStack  # noqa: E402

N_CORES = 8
B = 16384
B_LOCAL = B // N_CORES  # 2048
D = 1024
H = 2048
SBLK = 512              # batch columns per superblock
NSBLK = B_LOCAL // SBLK  # 4
NHT = H // 128           # 16
EPS = 1e-5
# (feature offset in obs, K_in) per group; g0 is the concat [t0,t1] group
GROUPS = [(0, 512)] + [(512 + i * 256, 256) for i in range(6)]

F32 = mybir.dt.float32
F32R = mybir.dt.float32r
MM_DT = F32R  # full-rate fp32 PE path
AF = mybir.ActivationFunctionType
ALU = mybir.AluOpType

_nc_cache = {}


def _build(normw_is_one=True, normb_is_zero=True):
    nc = bacc.Bacc("TRN2", target_bir_lowering=False, debug=False, enable_asserts=False)

    xt_d = nc.dram_tensor("xt", [2048, B_LOCAL], F32R, kind="ExternalInput").ap()
    w13_d = nc.dram_tensor("w13t", [4096, H], F32R, kind="ExternalInput").ap()
    w2_d = nc.dram_tensor("w2t", [7 * H, D], F32R, kind="ExternalInput").ap()
    nw_d = nc.dram_tensor("normw", [D], F32, kind="ExternalInput").ap()
    nb_d = nc.dram_tensor("normb", [D], F32, kind="ExternalInput").ap()
    te_d = nc.dram_tensor("te", [7, D], F32, kind="ExternalInput").ap()
    out_d = nc.dram_tensor("out", [B_LOCAL, 7, D], F32, kind="ExternalOutput").ap()

    with tile.TileContext(nc) as tc, ExitStack() as ctx:
        w13_pool = ctx.enter_context(tc.tile_pool(name="w13", bufs=8))
        w2_pool = ctx.enter_context(tc.tile_pool(name="w2", bufs=16))
        xt_pool = ctx.enter_context(tc.tile_pool(name="xt", bufs=2))
        act_pool = ctx.enter_context(tc.tile_pool(name="act", bufs=16))
        tok_pool = ctx.enter_context(tc.tile_pool(name="tok", bufs=2))
        sq_pool = ctx.enter_context(tc.tile_pool(name="sq", bufs=1))
        sg_pool = ctx.enter_context(tc.tile_pool(name="sg", bufs=2))
        ot_pool = ctx.enter_context(tc.tile_pool(name="ot", bufs=2))
        te_pool = ctx.enter_context(tc.tile_pool(name="teb", bufs=1))
        tetmp_pool = ctx.enter_context(tc.tile_pool(name="tetmp", bufs=1))
        small_pool = ctx.enter_context(tc.tile_pool(name="small", bufs=4))
        const_pool = ctx.enter_context(tc.tile_pool(name="consts", bufs=1))
        psA = ctx.enter_context(tc.tile_pool(name="psA", bufs=4, space="PSUM"))
        psB = ctx.enter_context(tc.tile_pool(name="psB", bufs=4, space="PSUM"))

        normw_b = None
        if not normw_is_one:
            normw_b = const_pool.tile([128, D], F32, tag="normw")
            nc.sync.dma_start(out=normw_b, in_=nw_d.partition_broadcast(128))
        normb_b = None
        if not normb_is_zero:
            normb_b = const_pool.tile([128, D], F32, tag="normb")
            nc.sync.dma_start(out=normb_b, in_=nb_d.partition_broadcast(128))
        eps_t = const_pool.tile([128, 1], F32, tag="eps")
        nc.vector.memset(eps_t, EPS)

        # Process a cheap rest-group first so g0's 16MB of weights stream in
        # under compute; row offsets of each group in the packed w13t tensor.
        GORDER = [1, 0, 2, 3, 4, 5, 6]
        W13_ROWS = [0, 1024, 1536, 2048, 2560, 3072, 3584]

        # first xt load issued ahead of all weight DMAs so PE can start early
        g_first, (foff0, kin0) = GORDER[0], GROUPS[GORDER[0]]
        pre0 = xt_pool.tile([128, kin0 // 128, SBLK], MM_DT, tag="xt")
        nc.sync.dma_start(
            out=pre0,
            in_=xt_d[foff0 : foff0 + kin0, 0:SBLK].rearrange("(t p) b -> p t b", p=128),
        )
        pre_xts0 = [pre0[:, k, :] for k in range(kin0 // 128)]

        # Deferred epilogue: the last batch-tile's norm chain of a superblock is
        # emitted a few hts into the NEXT superblock's L1, so its ACT/DVE ops
        # don't block the next superblock's silu/mul pipeline.
        pending = [None]

        def flush_pending():
            if pending[0] is not None:
                pending[0]()
                pending[0] = None

        for g in GORDER:
            foff, kin = GROUPS[g]
            nk = kin // 128
            w13_row = W13_ROWS[g]

            w1T = []
            w3T = []
            for k in range(nk):
                t = w13_pool.tile([128, H], MM_DT, tag="w13")
                nc.sync.dma_start(
                    out=t, in_=w13_d[w13_row + k * 128 : w13_row + (k + 1) * 128, :]
                )
                w1T.append(t)
            for k in range(nk):
                t = w13_pool.tile([128, H], MM_DT, tag="w13")
                nc.sync.dma_start(
                    out=t,
                    in_=w13_d[w13_row + kin + k * 128 : w13_row + kin + (k + 1) * 128, :],
                )
                w3T.append(t)

            w2T = []
            for ht in range(NHT):
                t = w2_pool.tile([128, D], MM_DT, tag="w2")
                nc.gpsimd.dma_start(
                    out=t, in_=w2_d[g * H + ht * 128 : g * H + (ht + 1) * 128, :]
                )
                w2T.append(t)

            bias_te = te_pool.tile([128, D], F32, tag="biaste")
            if normb_is_zero:
                nc.gpsimd.dma_start(out=bias_te, in_=te_d[g].partition_broadcast(128))
            else:
                te_tmp = tetmp_pool.tile([128, D], F32, tag="tetmp")
                nc.gpsimd.dma_start(out=te_tmp, in_=te_d[g].partition_broadcast(128))
                nc.vector.tensor_add(bias_te, te_tmp, normb_b)

            def load_xts(sblk, foff=foff, kin=kin, nk=nk):
                b0 = sblk * SBLK
                t = xt_pool.tile([128, nk, SBLK], MM_DT, tag="xt")
                nc.sync.dma_start(
                    out=t,
                    in_=xt_d[foff : foff + kin, b0 : b0 + SBLK].rearrange(
                        "(t p) b -> p t b", p=128
                    ),
                )
                return [t[:, k, :] for k in range(nk)]

            if g == g_first:
                xts_cur = pre_xts0
            else:
                xts_cur = load_xts(0)
            for sblk in range(NSBLK):
                b0 = sblk * SBLK
                xts = xts_cur
                xts_next = load_xts(sblk + 1) if sblk + 1 < NSBLK else None

                # L1: SwiGLU up-projections, feature-major output
                actT = []
                for ht in range(NHT):
                    hs = slice(ht * 128, (ht + 1) * 128)
                    pg = psA.tile([128, SBLK], F32, tag="pg")
                    pv = psA.tile([128, SBLK], F32, tag="pg")
                    for k in range(nk):
                        nc.tensor.matmul(
                            pg, w1T[k][:, hs], xts[k], start=(k == 0), stop=(k == nk - 1)
                        )
                    for k in range(nk):
                        nc.tensor.matmul(
                            pv, w3T[k][:, hs], xts[k], start=(k == 0), stop=(k == nk - 1)
                        )
                    sg = sg_pool.tile([128, SBLK], F32, tag="sg")
                    nc.scalar.activation(sg, pg, AF.Silu)
                    a = act_pool.tile([128, SBLK], MM_DT, tag="act")
                    nc.vector.tensor_mul(a, sg, pv)
                    actT.append(a)
                    if ht == 3:
                        flush_pending()

                # L2: down-projection to batch-major tok, fused RMSNorm + embed
                nbt = SBLK // 128
                for bt in range(nbt):
                    bs = slice(bt * 128, (bt + 1) * 128)
                    pts = []
                    for dh in range(2):
                        ds = slice(dh * 512, (dh + 1) * 512)
                        pt = psB.tile([128, 512], F32, tag="pt")
                        for ht in range(NHT):
                            nc.tensor.matmul(
                                pt,
                                actT[ht][:, bs],
                                w2T[ht][:, ds],
                                start=(ht == 0),
                                stop=(ht == NHT - 1),
                            )
                        pts.append(pt)

                    def epi(pts=pts, b0=b0, bt=bt, g=g, bias_te=bias_te):
                        parts = small_pool.tile([128, 2], F32, tag="parts")
                        tok = tok_pool.tile([128, D], F32, tag="tok")
                        for dh in range(2):
                            ds = slice(dh * 512, (dh + 1) * 512)
                            sq = sq_pool.tile([128, 512], F32, tag="sq")
                            nc.scalar.activation(
                                sq, pts[dh], AF.Square, accum_out=parts[:, dh : dh + 1]
                            )
                            nc.vector.tensor_copy(out=tok[:, ds], in_=pts[dh])
                        ssum = small_pool.tile([128, 1], F32, tag="ssum")
                        nc.vector.tensor_add(ssum, parts[:, 0:1], parts[:, 1:2])
                        rms = small_pool.tile([128, 1], F32, tag="rms")
                        nc.scalar.activation(
                            rms, ssum, AF.Sqrt, bias=eps_t[:, 0:1], scale=1.0 / D
                        )
                        rstd = small_pool.tile([128, 1], F32, tag="rstd")
                        nc.vector.reciprocal(rstd, rms)
                        ot = ot_pool.tile([128, D], F32, tag="ot")
                        nc.scalar.activation(ot, tok, AF.Copy, scale=rstd[:, 0:1])
                        if normw_b is not None:
                            nc.vector.tensor_mul(ot, ot, normw_b)
                        nc.vector.tensor_add(ot, ot, bias_te)
                        nc.gpsimd.dma_start(
                            out=out_d[b0 + bt * 128 : b0 + (bt + 1) * 128, g, :], in_=ot
                        )

                    if bt < nbt - 1:
                        epi()
                    else:
                        pending[0] = epi
                xts_cur = xts_next
        flush_pending()

    nc.compile()
    return nc


def _get_nc(normw_is_one, normb_is_zero):
    key = (normw_is_one, normb_is_zero)
    if key not in _nc_cache:
        _nc_cache[key] = _build(*key)
    return _nc_cache[key]


def _host_pack(inputs):
    f = np.float32
    obs = np.asarray(inputs["obs"], f)
    w1_g0 = np.asarray(inputs["w1_g0"], f)
    w3_g0 = np.asarray(inputs["w3_g0"], f)
    w2_g0 = np.asarray(inputs["w2_g0"], f)
    w1_r = np.asarray(inputs["w1_r"], f)
    w3_r = np.asarray(inputs["w3_r"], f)
    w2_r = np.asarray(inputs["w2_r"], f)

    w13_parts = [w1_g0.T, w3_g0.T]
    for i in range(6):
        w13_parts.append(w1_r[i].T)
        w13_parts.append(w3_r[i].T)
    w13t = np.ascontiguousarray(np.concatenate(w13_parts, axis=0))  # [4096, 2048]

    w2_parts = [w2_g0.T] + [w2_r[i].T for i in range(6)]
    w2t = np.ascontiguousarray(np.concatenate(w2_parts, axis=0))  # [14336, 1024]

    common = {
        "w13t": w13t,
        "w2t": w2t,
        "normw": np.ascontiguousarray(np.asarray(inputs["norm_w"], f)),
        "normb": np.ascontiguousarray(np.asarray(inputs["norm_b"], f)),
        "te": np.ascontiguousarray(np.asarray(inputs["term_embed"], f)),
    }
    in_maps = []
    for c in range(N_CORES):
        m = dict(common)
        m["xt"] = np.ascontiguousarray(obs[c * B_LOCAL : (c + 1) * B_LOCAL].T)
        in_maps.append(m)
    return in_maps


def run(inputs, trace=False, **kw):
    normw_is_one = bool(np.all(np.asarray(inputs["norm_w"]) == 1.0))
    normb_is_zero = bool(np.all(np.asarray(inputs["norm_b"]) == 0.0))
    nc = _get_nc(normw_is_one, normb_is_zero)
    in_maps = _host_pack(inputs)
    res = bass_utils.run_bass_kernel_spmd(
        nc, in_maps, core_ids=list(range(N_CORES)), trace=trace, **kw
    )
    out = np.concatenate([r["out"] for r in res.results], axis=0)
    return out, res


def kernel(**inputs):
    out, _ = run(inputs, trace=False)
    return out
